# revision 4
# baseline (speedup 1.0000x reference)
"""Trainium2 Bass kernel for nn_Conv2dSubsamplingFUSED (4 conv towers + SMoE top-2).

Sharding: data-parallel over batch B=8 -> 1 item per NeuronCore, no collectives.

Per-core pipeline (all shapes per batch item):
  conv1 (1->256, 3x3 s2)   : single K=27 matmul per tile: bf16x2 3-pass folded
                             into the contraction dim ([Whi;Wlo;Whi] x [Phi;Phi;Plo])
  conv2 (256->256, 3x3 s2) : 9 taps x 2 ksub x {hi@hi, lo@hi, hi@lo} bf16 matmuls
                             accumulated in PSUM (fp32); h2 kept fp32 in SBUF
  gate                     : host-folds 16*lin_w^T @ gate_w^T -> (4864,8) per tower;
                             true-fp32 matmuls (tok,8) from fp32 h2 => exact top-2
  linear (4864->256)       : fp32 matmuls reading h2 in place
  experts (8x 1024->256)   : bf16 dense, top-2 combine on-chip via is_equal masks

Precision: conv chain error ~5e-6 (rel) keeps the MoE routing identical to the
fp32 reference (min top2/top3 gate gap for these inputs is 8.2e-6 abs, and the
gate path error is ~2e-6); expert path is bf16 (~2.4e-3 final rel L2).
"""

import math

import ml_dtypes
import numpy as np

BF16 = ml_dtypes.bfloat16

B, T, IDIM = 8, 2048, 80
ODIM, NREPR, NEXP = 256, 4, 8
T1, F1 = 1023, 39      # conv1 output
T2, F2 = 511, 19       # conv2 output
LIN_K = F2 * ODIM      # 4864
NKS_LIN = LIN_K // 128  # 38
SCALE = math.sqrt(ODIM)  # 16.0

CHUNK = 26             # conv2 output rows per psum tile (N = 26*19 = 494)
C1SUB = 13             # conv1 rows per psum tile (N = 13*39 = 507)
SB = 256               # superblock tokens (linear N >= 256 for fp32r fast path)


def _split_bf16(a):
    hi = np.asarray(a, np.float32).astype(BF16)
    lo = (np.asarray(a, np.float32) - hi.astype(np.float32)).astype(BF16)
    return hi, lo


def _sb_sizes(t2):
    sizes = []
    t = t2
    while t > 0:
        s = min(SB, t)
        sizes.append(s)
        t -= s
    return sizes


def _chunks(n, c):
    out = []
    t = n
    while t > 0:
        s = min(c, t)
        out.append(s)
        t -= s
    return out


def _tok_tiles(t2):
    return _chunks(t2, 128)


def build_program(t_out=T2, nrepr=NREPR, debug=False):
    """Build the per-core Bass program. t_out/nrepr reducible for self-test."""
    import concourse.mybir as mybir
    import concourse.tile as tile
    from concourse import bacc

    f32 = mybir.dt.float32
    f32r = mybir.dt.float32r
    bf16 = mybir.dt.bfloat16

    t1_rows = 2 * t_out + 1          # conv1 rows needed
    sb_sizes = _sb_sizes(t_out)
    tok_tiles = _tok_tiles(t_out)
    n_tok_tiles = len(tok_tiles)

    nc = bacc.Bacc("TRN2", target_bir_lowering=False, debug=False, num_devices=8)

    # ---- DRAM inputs (per core) ----
    d_p27 = nc.dram_tensor("p27", (nrepr, 27, t1_rows, F1), bf16, kind="ExternalInput")
    d_w1s = nc.dram_tensor("w1s", (nrepr, 27, 2, 128), bf16, kind="ExternalInput")
    d_w2hi = nc.dram_tensor("w2hi", (nrepr, 128, 9, 2, 2, 128), bf16, kind="ExternalInput")
    d_w2lo = nc.dram_tensor("w2lo", (nrepr, 128, 9, 2, 2, 128), bf16, kind="ExternalInput")
    d_linT = nc.dram_tensor("linT", (nrepr, 128, NKS_LIN, 2, 128), f32, kind="ExternalInput")
    d_gwp = nc.dram_tensor("gwp", (128, nrepr, NKS_LIN, 8), f32, kind="ExternalInput")
    d_gb = nc.dram_tensor("gb", (128, 8), f32, kind="ExternalInput")
    d_b1 = nc.dram_tensor("b1", (128, nrepr, 2, 1), f32, kind="ExternalInput")
    d_b2 = nc.dram_tensor("b2", (128, nrepr, 2, 1), f32, kind="ExternalInput")
    d_lb = nc.dram_tensor("lb", (128, nrepr, 2, 1), f32, kind="ExternalInput")
    d_expT = nc.dram_tensor("expT", (128, NEXP, 2 * nrepr, 256), bf16, kind="ExternalInput")
    d_expb = nc.dram_tensor("expb", (1, NEXP, 256), bf16, kind="ExternalInput")

    d_out = nc.dram_tensor("outp", (t_out, 256), f32, kind="ExternalOutput")
    d_gout = nc.dram_tensor("gout", (128, n_tok_tiles, 8), f32, kind="ExternalOutput")
    if debug:
        d_h2dbg = nc.dram_tensor("h2dbg", (128, 2, sb_sizes[0], F2), f32, kind="ExternalOutput")
        d_xdbg = nc.dram_tensor("xdbg", (128, 2 * nrepr, t_out), bf16, kind="ExternalOutput")

    with tile.TileContext(nc) as tc:
        with (
            tc.tile_pool(name="singles", bufs=1) as singles,
            tc.tile_pool(name="wts", bufs=1) as wts,
            tc.tile_pool(name="patches", bufs=2) as patches,
            tc.tile_pool(name="h1p", bufs=2) as h1p,
            tc.tile_pool(name="h2p", bufs=1) as h2p,
            tc.tile_pool(name="small", bufs=2) as small,
            tc.tile_pool(name="c1tp", bufs=2) as c1tp,
            tc.tile_pool(name="outs", bufs=2) as outs,
            tc.tile_pool(name="ps_c1", bufs=2, space="PSUM") as ps_c1,
            tc.tile_pool(name="ps_c2", bufs=2, space="PSUM") as ps_c2,
            tc.tile_pool(name="ps_sm", bufs=4, space="PSUM") as ps_sm,
        ):
            # ---- persistent tiles ----
            expT_sb = singles.tile([128, NEXP, 2 * nrepr, 256], bf16)
            nc.sync.dma_start(expT_sb[:], d_expT[:])
            expb_sb = singles.tile([1, NEXP, 256], bf16)
            nc.sync.dma_start(expb_sb[:], d_expb[:])
            ones_sb = singles.tile([1, 128], bf16)
            nc.vector.memset(ones_sb[:], 1.0)
            gwp_sb = singles.tile([128, nrepr, NKS_LIN, 8], f32)
            nc.sync.dma_start(gwp_sb[:], d_gwp[:])
            gb_sb = singles.tile([128, 8], f32)
            nc.sync.dma_start(gb_sb[:], d_gb[:])
            b1_sb = singles.tile([128, nrepr, 2, 1], f32)
            nc.sync.dma_start(b1_sb[:], d_b1[:])
            b2_sb = singles.tile([128, nrepr, 2, 1], f32)
            nc.sync.dma_start(b2_sb[:], d_b2[:])
            lb_sb = singles.tile([128, nrepr, 2, 1], f32)
            nc.sync.dma_start(lb_sb[:], d_lb[:])
            x_sb = singles.tile([128, 2 * nrepr, t_out], bf16)
            g_sb = singles.tile([128, n_tok_tiles, 8], f32)
            for tt in range(n_tok_tiles):
                nc.vector.tensor_copy(g_sb[:, tt, :], gb_sb[:])

            for r in range(nrepr):
                # ---- per-tower weights ----
                w1_sb = wts.tile([27, 2, 128], bf16, tag="w1")
                nc.sync.dma_start(w1_sb[:], d_w1s[r])
                w2hi_sb = wts.tile([128, 9, 2, 2, 128], bf16, tag="w2hi")
                nc.sync.dma_start(w2hi_sb[:], d_w2hi[r])
                w2lo_sb = wts.tile([128, 9, 2, 2, 128], bf16, tag="w2lo")
                nc.sync.dma_start(w2lo_sb[:], d_w2lo[r])

                sb_t0 = 0
                for sbi, sb_len in enumerate(sb_sizes):
                    h2_sb = h2p.tile([128, 2, SB, F2], f32, tag="h2")
                    ch_t0 = 0  # token offset within superblock
                    for ct in _chunks(sb_len, CHUNK):
                        tok0 = sb_t0 + ch_t0  # global first token of chunk
                        rows = 2 * ct + 1     # conv1 rows needed
                        row0 = 2 * tok0       # first conv1 row (== patch row)

                        # ---- conv1 for this chunk ----
                        pat = patches.tile([27, CHUNK * 2 + 1, F1], bf16, tag="pat")
                        nc.sync.dma_start(
                            pat[:, :rows, :], d_p27[r][:, row0:row0 + rows, :]
                        )
                        h1hi = h1p.tile([128, 2, CHUNK * 2 + 1, F1], bf16, tag="h1hi")
                        h1lo = h1p.tile([128, 2, CHUNK * 2 + 1, F1], bf16, tag="h1lo")
                        r0 = 0
                        for sc in _chunks(rows, C1SUB):
                            for ms in range(2):
                                p1 = ps_c1.tile([128, C1SUB, F1], f32, tag="ps1")
                                nc.tensor.matmul(
                                    p1[:, :sc, :],
                                    w1_sb[:, ms, :],
                                    pat[:, r0:r0 + sc, :],
                                    start=True, stop=True,
                                )
                                # hi = bf16(relu(z + b)) on ACT
                                nc.scalar.activation(
                                    h1hi[:, ms, r0:r0 + sc, :],
                                    p1[:, :sc, :],
                                    mybir.ActivationFunctionType.Relu,
                                    bias=b1_sb[:, r, ms],
                                )
                                # lo = bf16(relu(z + b) - hi) on DVE
                                tmp = c1tp.tile([128, C1SUB, F1], f32, tag="c1tmp")
                                nc.vector.tensor_scalar(
                                    out=tmp[:, :sc, :], in0=p1[:, :sc, :],
                                    scalar1=b1_sb[:, r, ms], scalar2=0.0,
                                    op0=mybir.AluOpType.add, op1=mybir.AluOpType.max,
                                )
                                nc.vector.tensor_tensor(
                                    out=h1lo[:, ms, r0:r0 + sc, :],
                                    in0=tmp[:, :sc, :],
                                    in1=h1hi[:, ms, r0:r0 + sc, :],
                                    op=mybir.AluOpType.subtract,
                                )
                            r0 += sc

                        # ---- conv2 for this chunk ----
                        for ms in range(2):
                            p2 = ps_c2.tile([128, CHUNK, F2], f32, tag="ps2")
                            first = True
                            n_mm = 9 * 2 * 3
                            i_mm = 0
                            for kt in range(3):
                                for kf in range(3):
                                    tap = kt * 3 + kf
                                    for ks in range(2):
                                        for w_t, a_t in (
                                            (w2hi_sb, h1hi), (w2lo_sb, h1hi), (w2hi_sb, h1lo),
                                        ):
                                            i_mm += 1
                                            nc.tensor.matmul(
                                                p2[:, :ct, :],
                                                w_t[:, tap, ks, ms, :],
                                                a_t[:, ks, kt:kt + 2 * ct - 1:2, kf:kf + 2 * F2 - 1:2],
                                                start=first, stop=(i_mm == n_mm),
                                            )
                                            first = False
                            # h2 = relu(z + b) fp32
                            nc.scalar.activation(
                                h2_sb[:, ms, ch_t0:ch_t0 + ct, :],
                                p2[:, :ct, :],
                                mybir.ActivationFunctionType.Relu,
                                bias=b2_sb[:, r, ms],
                            )
                        ch_t0 += ct

                    # ---- gate for this superblock (true fp32) ----
                    tt0 = sb_t0 // 128
                    gt0 = 0
                    while gt0 < sb_len:
                        tt = (sb_t0 + gt0) // 128
                        ntok = min(128, sb_len - gt0)
                        pg = ps_sm.tile([128, 8], f32, tag="ps_small")
                        for j in range(NKS_LIN):
                            f, chh = j // 2, j % 2
                            nc.tensor.matmul(
                                pg[:ntok, :],
                                h2_sb[:, chh, gt0:gt0 + ntok, f],
                                gwp_sb[:, r, j, :],
                                start=(j == 0), stop=(j == NKS_LIN - 1),
                            )
                        nc.vector.tensor_tensor(
                            out=g_sb[:ntok, tt, :], in0=g_sb[:ntok, tt, :],
                            in1=pg[:ntok, :], op=mybir.AluOpType.add,
                        )
                        gt0 += ntok

                    # ---- linear for this superblock (fp32) ----
                    for ms in range(2):
                        lin_sb = wts.tile([128, NKS_LIN, 128], f32, tag="lin")
                        nc.sync.dma_start(lin_sb[:], d_linT[r][:, :, ms, :])
                        px = ps_sm.tile([128, SB], f32, tag="ps_small")
                        for j in range(NKS_LIN):
                            f, chh = j // 2, j % 2
                            nc.tensor.matmul(
                                px[:, :sb_len],
                                lin_sb[:, j, :],
                                h2_sb[:, chh, :sb_len, f],
                                start=(j == 0), stop=(j == NKS_LIN - 1),
                            )
                        nc.vector.tensor_scalar(
                            out=x_sb[:, 2 * r + ms, sb_t0:sb_t0 + sb_len],
                            in0=px[:, :sb_len], scalar1=lb_sb[:, r, ms],
                            scalar2=None, op0=mybir.AluOpType.add,
                        )
                    if debug and r == 0 and sbi == 0:
                        nc.sync.dma_start(d_h2dbg[:], h2_sb[:, :, :sb_sizes[0], :])
                    sb_t0 += sb_len

            # ---- MoE combine ----
            nc.sync.dma_start(d_gout[:], g_sb[:])
            if debug:
                nc.sync.dma_start(d_xdbg[:], x_sb[:])
            t0 = 0
            for tt, ntok in enumerate(tok_tiles):
                g = g_sb[:ntok, tt, :]
                m1 = small.tile([128, 1], f32, tag="m1")
                nc.vector.tensor_reduce(m1[:ntok], g, axis=mybir.AxisListType.X,
                                        op=mybir.AluOpType.max)
                eq1 = small.tile([128, 8], f32, tag="eq1")
                nc.vector.tensor_scalar(out=eq1[:ntok], in0=g, scalar1=m1[:ntok],
                                        scalar2=None, op0=mybir.AluOpType.is_equal)
                gm = small.tile([128, 8], f32, tag="gm")
                nc.vector.tensor_scalar(out=gm[:ntok], in0=eq1[:ntok], scalar1=-1e30,
                                        scalar2=None, op0=mybir.AluOpType.mult)
                nc.vector.tensor_tensor(out=gm[:ntok], in0=gm[:ntok], in1=g,
                                        op=mybir.AluOpType.add)
                m2 = small.tile([128, 1], f32, tag="m2")
                nc.vector.tensor_reduce(m2[:ntok], gm[:ntok], axis=mybir.AxisListType.X,
                                        op=mybir.AluOpType.max)
                eq2 = small.tile([128, 8], f32, tag="eq2")
                nc.vector.tensor_scalar(out=eq2[:ntok], in0=gm[:ntok], scalar1=m2[:ntok],
                                        scalar2=None, op0=mybir.AluOpType.is_equal)
                d = small.tile([128, 1], f32, tag="d")
                nc.vector.tensor_tensor(out=d[:ntok], in0=m2[:ntok], in1=m1[:ntok],
                                        op=mybir.AluOpType.subtract)
                w2c = small.tile([128, 1], f32, tag="w2c")
                nc.scalar.activation(w2c[:ntok], d[:ntok],
                                     mybir.ActivationFunctionType.Sigmoid)
                w1c = small.tile([128, 1], f32, tag="w1c")
                nc.scalar.activation(w1c[:ntok], d[:ntok],
                                     mybir.ActivationFunctionType.Sigmoid, scale=-1.0)
                coef = small.tile([128, 8], f32, tag="coef")
                nc.vector.tensor_scalar(out=coef[:ntok], in0=eq1[:ntok],
                                        scalar1=w1c[:ntok], scalar2=None,
                                        op0=mybir.AluOpType.mult)
                ctmp = small.tile([128, 8], f32, tag="ctmp")
                nc.vector.tensor_scalar(out=ctmp[:ntok], in0=eq2[:ntok],
                                        scalar1=w2c[:ntok], scalar2=None,
                                        op0=mybir.AluOpType.mult)
                nc.vector.tensor_tensor(out=coef[:ntok], in0=coef[:ntok],
                                        in1=ctmp[:ntok], op=mybir.AluOpType.add)

                acc = outs.tile([128, 256], f32, tag="acc")
                etmp = outs.tile([128, 256], f32, tag="etmp")
                for e in range(NEXP):
                    pe = ps_sm.tile([128, 256], f32, tag="ps_small")
                    for ks in range(2 * nrepr):
                        nc.tensor.matmul(
                            pe[:ntok, :],
                            x_sb[:, ks, t0:t0 + ntok],
                            expT_sb[:, e, ks, :],
                            start=(ks == 0), stop=False,
                        )
                    nc.tensor.matmul(
                        pe[:ntok, :], ones_sb[:, :ntok], expb_sb[:, e, :],
                        start=False, stop=True,
                    )
                    if e == 0:
                        nc.vector.tensor_scalar(out=acc[:ntok], in0=pe[:ntok],
                                                scalar1=coef[:ntok, 0:1], scalar2=None,
                                                op0=mybir.AluOpType.mult)
                    else:
                        nc.vector.tensor_scalar(out=etmp[:ntok], in0=pe[:ntok],
                                                scalar1=coef[:ntok, e:e + 1], scalar2=None,
                                                op0=mybir.AluOpType.mult)
                        nc.vector.tensor_tensor(out=acc[:ntok], in0=acc[:ntok],
                                                in1=etmp[:ntok], op=mybir.AluOpType.add)
                nc.sync.dma_start(d_out[t0:t0 + ntok, :], acc[:ntok])
                t0 += ntok

    nc.compile()
    return nc


# ---------------- host-side packing ----------------

def pack_weights(conv1_w, conv1_b, conv2_w, conv2_b, lin_w, lin_b,
                 gate_w, gate_b, exp_w, exp_b, nrepr=NREPR):
    conv1_w = np.asarray(conv1_w, np.float32)
    conv2_w = np.asarray(conv2_w, np.float32)
    lin_w = np.asarray(lin_w, np.float32)
    gate_w = np.asarray(gate_w, np.float32)
    exp_w = np.asarray(exp_w, np.float32)

    # conv1: stacked [Whi; Wlo; Whi] over taps -> (r, 27, ms, 128)
    w9 = conv1_w[:, :, 0].reshape(nrepr, ODIM, 9).transpose(0, 2, 1)  # (r, 9, 256)
    w9hi, w9lo = _split_bf16(w9)
    w1s = np.concatenate([w9hi, w9lo, w9hi], axis=1)  # (r, 27, 256)
    w1s = w1s.reshape(nrepr, 27, 2, 128)

    # conv2 lhsT: [r][k][tap][ks][ms][m] = W2[r, 128ms+m, 128ks+k, kt, kf]
    w2 = conv2_w.reshape(nrepr, 2, 128, 2, 128, 3, 3)  # [r][ms][m][ks][k][kt][kf]
    w2 = w2.transpose(0, 4, 5, 6, 3, 1, 2)             # [r][k][kt][kf][ks][ms][m]
    w2 = w2.reshape(nrepr, 128, 9, 2, 2, 128)
    w2hi, w2lo = _split_bf16(w2)

    # linear lhsT: [r][k][j][ms][m] = 16*lin_w[r, 128ms+m, 128j+k]
    lw = (SCALE * lin_w).reshape(nrepr, 2, 128, NKS_LIN, 128)  # [r][ms][m][j][k]
    linT = np.ascontiguousarray(lw.transpose(0, 4, 3, 1, 2), np.float32)

    # folded gate: Gwp[r] = (16*lin_w[r]).T @ gate_w[:, 256r:256r+256].T  (4864, 8)
    gwp = np.zeros((128, nrepr, NKS_LIN, 8), np.float32)
    for r in range(nrepr):
        g = (SCALE * lin_w[r].astype(np.float64)).T @ \
            gate_w[:, r * ODIM:(r + 1) * ODIM].astype(np.float64).T
        gwp[:, r] = g.reshape(NKS_LIN, 128, 8).transpose(1, 0, 2).astype(np.float32)

    gb = np.broadcast_to(np.asarray(gate_b, np.float32), (128, 8)).copy()

    def bias_pack(b):
        b = np.asarray(b, np.float32).reshape(nrepr, 2, 128)
        return np.ascontiguousarray(b.transpose(2, 0, 1))[..., None].copy()

    b1 = bias_pack(conv1_b)
    b2 = bias_pack(conv2_b)
    lb = bias_pack(SCALE * np.asarray(lin_b, np.float32))

    # experts: expT[k][e][ks][n] = exp_w[e, n, 128ks+k]
    et = exp_w.reshape(NEXP, 256, 2 * nrepr, 128)  # [e][n][ks][k]
    expT = np.ascontiguousarray(et.transpose(3, 0, 2, 1)).astype(BF16)
    expb = np.asarray(exp_b, np.float32).reshape(1, NEXP, 256).astype(BF16)

    return dict(w1s=w1s.astype(BF16), w2hi=w2hi, w2lo=w2lo, linT=linT, gwp=gwp,
                gb=gb, b1=b1, b2=b2, lb=lb, expT=expT, expb=expb)


def pack_patches(xs, t1_rows=T1, nrepr=NREPR):
    """xs: list of nrepr arrays (T, IDIM) for ONE batch item -> (nrepr,27,t1,39) bf16."""
    p27 = np.empty((nrepr, 27, t1_rows, F1), BF16)
    for r in range(nrepr):
        x = np.asarray(xs[r], np.float32)
        pat = np.empty((9, t1_rows, F1), np.float32)
        for kt in range(3):
            for kf in range(3):
                pat[kt * 3 + kf] = x[kt:kt + 2 * t1_rows - 1:2, kf:kf + 2 * F1 - 1:2]
        hi, lo = _split_bf16(pat)
        p27[r, 0:9] = hi
        p27[r, 9:18] = hi
        p27[r, 18:27] = lo
    return p27


def _pos_emb(t, d):
    pos = np.arange(t - 1, -t, -1, dtype=np.float32)[:, None]
    div = np.exp(np.arange(0, d, 2, dtype=np.float32) * (-math.log(10000.0) / d))
    pe = np.zeros((2 * t - 1, d), np.float32)
    pe[:, 0::2] = np.sin(pos * div)
    pe[:, 1::2] = np.cos(pos * div)
    return pe[None]


_PROGRAM = None


def _get_program():
    global _PROGRAM
    if _PROGRAM is None:
        _PROGRAM = build_program()
    return _PROGRAM


def make_in_maps(x0, x1, x2, x3, conv1_w, conv1_b, conv2_w, conv2_b,
                 lin_w, lin_b, gate_w, gate_b, exp_w, exp_b):
    wts = pack_weights(conv1_w, conv1_b, conv2_w, conv2_b, lin_w, lin_b,
                       gate_w, gate_b, exp_w, exp_b)
    in_maps = []
    for b in range(B):
        m = dict(wts)
        m["p27"] = pack_patches([np.asarray(x)[b] for x in (x0, x1, x2, x3)])
        in_maps.append(m)
    return in_maps


def kernel(x0, x1, x2, x3, x_mask, conv1_w, conv1_b, conv2_w, conv2_b,
           lin_w, lin_b, gate_w, gate_b, exp_w, exp_b):
    from concourse.bass_utils import run_bass_kernel_spmd

    nc = _get_program()
    in_maps = make_in_maps(x0, x1, x2, x3, conv1_w, conv1_b, conv2_w, conv2_b,
                           lin_w, lin_b, gate_w, gate_b, exp_w, exp_b)
    res = run_bass_kernel_spmd(nc, in_maps, core_ids=list(range(B)))
    out = np.stack([res.results[c]["outp"] for c in range(B)])
    pos_emb = _pos_emb(T2, ODIM)
    x_mask = np.asarray(x_mask)
    mask = x_mask[:, :, :-2:2][:, :, :-2:2]
    return out, pos_emb, mask
